# revision 7
# baseline (speedup 1.0000x reference)
"""Trainium2 Bass kernel for the pairwise adjacency layer.

Reference math (B=1024 points, D=128 dims):
    a   = dc_param[0]
    e   = exp(1 - dc)                                  # [B, D]
    den[i,j] = mean_d((1-a)*(x[i]-x[j])**2 + a*e[i]*e[j])
    out = 1/den off-diagonal, 1.0 on the diagonal      # [B, B]

Expansion used on-chip (no [B,B,D] tensor ever materializes), with the
runtime scalars folded into the matmul operands so PSUM accumulates den
directly and the reciprocal is the last math op:
    den = (-2c1 x_slab)^T @ x        (Gram term)
        + (c1 ones)^T     @ x2       (row broadcast of r_j)
        + (x2_slab)^T     @ c1*ones  (column broadcast of r_i)
        + (c3 e_slab)^T   @ e        (exp cross term)
    c1 = (1-a)/D, c3 = a/D, x2 = x*x, r = sum_d x^2
followed by one DVE reciprocal_approx_fast (~18 correct bits) and a
gpsimd affine_select stamping the diagonal to 1.0.

Sharding: pure output-row-parallel over 8 NeuronCores. Core c computes
output rows [c*128, (c+1)*128). Inputs x^T/dc^T are replicated to every
core; each core's copy has its columns rotated left by c*128 so the
diagonal block is always local columns 0:128 (SPMD-uniform diagonal
stamp); the host unshard rolls each slab back.

Engine/queue plan (from NTFF trace analysis; per-DMA issue ~0.65us and
issue-to-consumable ~2.3-2.7us pipe latency dominate the body):
  - inputs are packed per column-half on the host ([x_half | dc_half],
    [128,1024] bf16) so each half is ONE DMA; half 0 rides the SP HWDGE
    ring, half 1 the ACT ring - the two halves stream in parallel.
  - gpsimd (otherwise idle) computes the runtime-scalar columns and the
    small scaled slabs; DVE keeps only x^2, the c1*ones moving operand
    and the two reciprocals; ACT does the two exps.
  - ~56 tiny warm-up matmuls into an unread PSUM tile keep the PE busy
    from ~1.5us so the HAM clock gate (cold 1.2 GHz) flips to 2.4 GHz
    around the time the real matmuls stream.
  - output halves go out on the two rings (half 0 on ACT, half 1 on SP)
    as soon as their reciprocal (and the diagonal stamp for half 0) is
    done.
The NRT execution wrapper adds ~1.2us of preamble and ~8us of teardown
(per-engine zeroing of all 256 semaphores) inside the measured window;
that part is runtime-injected and unavoidable.

Matmul operands are bf16 (full-rate PE streaming; fp32 PSUM
accumulation; rel err ~2e-3), everything else fp32.
"""

import ml_dtypes
import numpy as np

import concourse.tile as tile
from concourse import bacc, mybir
from concourse.bass_utils import run_bass_kernel_spmd
from concourse.tile_rust import add_dep_helper

B = 1024          # number of points
D = 128           # feature dim
NCORES = 8
ROWS = B // NCORES  # output rows per core = 128
H = 512             # column half
F32 = mybir.dt.float32
BF16 = mybir.dt.bfloat16
AF = mybir.ActivationFunctionType
NWARM = 32          # PE warm-up matmuls (HAM un-throttle), N=64 each


def build_nc():
    nc = bacc.Bacc(None)
    # in{h} = [ xT half h | dcT half h ]  packed on the host
    in0 = nc.declare_dram_parameter("in0", [D, 2 * H], BF16, isOutput=False)
    in1 = nc.declare_dram_parameter("in1", [D, 2 * H], BF16, isOutput=False)
    apar = nc.declare_dram_parameter("apar", [D, 1], F32, isOutput=False)
    out = nc.declare_dram_parameter("out", [ROWS, B], F32, isOutput=True)

    with tile.TileContext(nc) as tc:
        with (
            tc.tile_pool(name="big", bufs=1) as big,
            tc.tile_pool(name="small", bufs=1) as small,
            tc.tile_pool(name="ps", bufs=1, space="PSUM") as ps,
        ):
            IN = [big.tile([D, 2 * H], BF16, name=f"IN{h}", tag=f"IN{h}")
                  for h in range(2)]
            ET = [big.tile([D, H], BF16, name=f"ET{h}", tag=f"ET{h}") for h in range(2)]
            X2 = [big.tile([D, H], BF16, name=f"X2{h}", tag=f"X2{h}") for h in range(2)]
            SIM = [big.tile([ROWS, H], F32, name=f"SIM{h}", tag=f"SIM{h}") for h in range(2)]
            C1ONESB = big.tile([D, H], BF16, tag="C1ONESB")
            XSC = small.tile([D, ROWS], BF16, tag="XSC")
            ESC = small.tile([D, ROWS], BF16, tag="ESC")
            C1ONES = small.tile([D, ROWS], BF16, tag="C1ONES")
            ONESB = small.tile([D, H], BF16, tag="ONESB")
            SA = small.tile([D, 1], F32, tag="SA")
            QS = small.tile([D, 3], F32, tag="QS")
            OB = small.tile([D, 1], F32, tag="OB")
            PS = [ps.tile([ROWS, H], F32, name=f"PS{h}", tag=f"PS{h}") for h in range(2)]
            PSW = ps.tile([ROWS, 64], F32, tag="PSW")

            XB = [IN[h][:, 0:H] for h in range(2)]
            DCT = [IN[h][:, H:2 * H] for h in range(2)]

            # ---- input DMAs: one per half, split across the two HWDGE
            # rings so they stream in parallel ----
            nc.sync.dma_start(IN[0][:], in0[:, :])
            nc.scalar.dma_start(SA[:], apar[:, :])
            nc.scalar.dma_start(IN[1][:], in1[:, :])

            # constants
            nc.vector.memset(ONESB[:], 1.0)
            nc.vector.memset(OB[:], 1.0)

            # ---- PE warm-up: small dummy matmuls into an unread PSUM
            # tile, back-to-back so the HAM activity window stays hot ----
            i_warm = []
            for k in range(NWARM):
                i_warm.append(nc.tensor.matmul(PSW[:], ONESB[:, 0:ROWS],
                                               ONESB[:, 0:64],
                                               start=True, stop=True))

            # ---- runtime scalars + small slabs on gpsimd (idle) ----
            M = mybir.AluOpType.mult
            A = mybir.AluOpType.add
            nc.gpsimd.tensor_scalar(QS[:, 0:1], SA[:, 0:1],
                                    -1.0 / D, 1.0 / D, M, A)     # c1
            nc.gpsimd.tensor_scalar(QS[:, 1:2], SA[:, 0:1],
                                    2.0 / D, -2.0 / D, M, A)     # -2c1
            nc.gpsimd.tensor_scalar_mul(QS[:, 2:3], SA[:, 0:1],
                                        1.0 / D)                 # c3
            nc.gpsimd.tensor_scalar_mul(C1ONES[:], ONESB[:, 0:ROWS],
                                        QS[:, 0:1])
            i_xsc = nc.gpsimd.tensor_scalar_mul(XSC[:], XB[0][:, 0:ROWS],
                                                QS[:, 1:2])

            # ---- DVE: c1*ones moving operand, x^2 halves ----
            i_c1ob = nc.vector.tensor_scalar_mul(C1ONESB[:], ONESB[:],
                                                 QS[:, 0:1])
            i_x2 = [None, None]
            i_x2[0] = nc.vector.tensor_mul(X2[0][:], XB[0][:], XB[0][:])
            i_x2[1] = nc.vector.tensor_mul(X2[1][:], XB[1][:], XB[1][:])

            # ---- ACT: e = exp(1 - dc) -> bf16 ----
            i_exp = [None, None]
            for h in range(2):
                i_exp[h] = nc.scalar.activation(ET[h][:], DCT[h][:], AF.Exp,
                                                bias=OB[:, 0:1], scale=-1.0)
            i_esc = nc.gpsimd.tensor_scalar_mul(ESC[:], ET[0][:, 0:ROWS],
                                                QS[:, 2:3])

            # ---- matmul accumulation groups, E term last ----
            i_mm = []
            i_recip = [None, None]
            for h in range(2):
                m1 = nc.tensor.matmul(PS[h][:], XSC[:], XB[h][:],
                                      start=True, stop=False)
                m3 = nc.tensor.matmul(PS[h][:], C1ONES[:], X2[h][:],
                                      start=False, stop=False)
                m4 = nc.tensor.matmul(PS[h][:], X2[0][:, 0:ROWS],
                                      C1ONESB[:, 0:H], start=False, stop=False)
                m2 = nc.tensor.matmul(PS[h][:], ESC[:], ET[h][:],
                                      start=False, stop=True)
                i_mm += [m1, m3, m4, m2]
                # ~5x faster than reciprocal(), ~18 correct bits - far inside
                # tolerance; den is bounded away from 0 (>= c3*min(e)^2 > 0)
                i_recip[h] = nc.vector.reciprocal_approx_fast(SIM[h][:],
                                                              PS[h][:])

            # PE program order: warm-ups then group 0 then group 1
            pe_order = i_warm + i_mm
            for a, b in zip(pe_order[1:], pe_order[:-1]):
                add_dep_helper(a.ins, b.ins, sync=False,
                               reason="PE program order")

            # diagonal := 1.0 (local columns 0:128 hold the diagonal block)
            nc.gpsimd.affine_select(
                SIM[0][:, 0:ROWS], SIM[0][:, 0:ROWS],
                pattern=[[1, ROWS]], compare_op=mybir.AluOpType.not_equal,
                fill=1.0, base=0, channel_multiplier=-1,
            )

            # out halves on separate rings so the stores overlap
            nc.scalar.dma_start(out[:, 0:H], SIM[0][:])
            nc.sync.dma_start(out[:, H:B], SIM[1][:])
    nc.finalize()
    return nc


def _prep_in_maps(x, dc, dc_param):
    x = np.ascontiguousarray(np.asarray(x, dtype=np.float32))
    dc = np.ascontiguousarray(np.asarray(dc, dtype=np.float32))
    a = np.full((128, 1), np.asarray(dc_param, dtype=np.float32).reshape(()),
                dtype=np.float32)
    xT = np.ascontiguousarray(x.T).astype(ml_dtypes.bfloat16)
    dcT = np.ascontiguousarray(dc.T).astype(ml_dtypes.bfloat16)
    in_maps = []
    for c in range(NCORES):
        sh = c * ROWS
        xr = np.roll(xT, -sh, axis=1)
        dr = np.roll(dcT, -sh, axis=1)
        in_maps.append({
            "in0": np.ascontiguousarray(
                np.concatenate([xr[:, 0:H], dr[:, 0:H]], axis=1)),
            "in1": np.ascontiguousarray(
                np.concatenate([xr[:, H:B], dr[:, H:B]], axis=1)),
            "apar": a,
        })
    return in_maps


def _unshard(results):
    out = np.empty((B, B), dtype=np.float32)
    for c in range(NCORES):
        sh = c * ROWS
        out[sh:sh + ROWS, :] = np.roll(results[c]["out"], sh, axis=1)
    return out


def kernel(x, dc, dc_param):
    nc = build_nc()
    res = run_bass_kernel_spmd(nc, _prep_in_maps(x, dc, dc_param),
                               list(range(NCORES)))
    return _unshard(res.results)


def _ensure_ntff_hook():
    """The agent image's ``antenv`` lacks ``axon_hooks``; synthesize it and
    register the ctypes NTFF-profiling hook so trace=True works."""
    import sys
    import types
    try:
        from antenv.axon_hooks import get_axon_ntff_profile_hook  # noqa: F401
        return
    except ImportError:
        pass
    mod = types.ModuleType("antenv.axon_hooks")
    mod._hook = None

    def set_axon_ntff_profile_hook(h):
        mod._hook = h

    def get_axon_ntff_profile_hook():
        return mod._hook

    mod.set_axon_ntff_profile_hook = set_axon_ntff_profile_hook
    mod.get_axon_ntff_profile_hook = get_axon_ntff_profile_hook
    sys.modules["antenv.axon_hooks"] = mod
    try:
        from trn_agent_boot.trn_boot import _ntff_profile_via_ctypes
        mod._hook = _ntff_profile_via_ctypes("/opt/axon/libaxon_pjrt.so")
    except Exception as e:  # degrade to no-trace
        print(f"ntff hook setup failed: {e}", file=sys.stderr)


def kernel_traced(x, dc, dc_param, reps=3):
    """Like kernel() but captures a neuron-profile trace; returns
    (output, best_exec_time_ns, trace_path). Runs `reps` times (the NEFF is
    compiled once and cached) and reports the fastest - exec time is noisy
    run-to-run (chip power state, co-tenants)."""
    _ensure_ntff_hook()
    nc = build_nc()
    in_maps = _prep_in_maps(x, dc, dc_param)
    best = None
    for _ in range(reps):
        res = run_bass_kernel_spmd(nc, in_maps, list(range(NCORES)),
                                   trace=True,
                                   trace_cores=list(range(NCORES)))
        print(f"  rep exec_time_ns: {res.exec_time_ns}")
        if best is None or (res.exec_time_ns or 1 << 60) < (
                best.exec_time_ns or 1 << 60):
            best = res
    trace_path = None
    if best.instructions_and_trace is not None:
        trace_path = best.instructions_and_trace[1]
    return _unshard(best.results), best.exec_time_ns, trace_path


# revision 8
# speedup vs baseline: 1.3047x; 1.3047x over previous
"""Trainium2 Bass kernel for the pairwise adjacency layer.

Reference math (B=1024 points, D=128 dims):
    a   = dc_param[0]
    e   = exp(1 - dc)                                  # [B, D]
    den[i,j] = mean_d((1-a)*(x[i]-x[j])**2 + a*e[i]*e[j])
    out = 1/den off-diagonal, 1.0 on the diagonal      # [B, B]

Expansion used on-chip (no [B,B,D] tensor ever materializes), with the
runtime scalars folded into the matmul stationaries so PSUM accumulates
den directly and the reciprocal is the last math op:
    den = (-2c1 x_slab)^T @ x       (Gram term)
        + (c1 ones)^T     @ x2      (row broadcast of r_j)
        + (c1 x2_slab)^T  @ ones    (column broadcast of r_i)
        + (c3 e_slab)^T   @ e       (exp cross term)
    c1 = (1-a)/D, c3 = a/D, x2 = x*x, r = sum_d x^2
followed by one DVE reciprocal_approx_fast (~18 correct bits) and a
gpsimd affine_select stamping the diagonal to 1.0.

Sharding: pure output-row-parallel over 8 NeuronCores. Core c computes
output rows [c*128, (c+1)*128). Inputs x^T/dc^T are replicated to every
core (on-chip collectives bounce through HBM and have a ~10us/step
firmware floor - far worse than re-reading 1MB per core). Each core's
copy has its columns rotated left by c*128 so that the diagonal block of
the output is always local columns 0:128, making the diagonal stamp an
SPMD-uniform affine_select; the host unshard rolls each slab back.

Lessons baked in from NTFF traces of prior variants:
  - per-DMA issue ~0.65us + issue-to-consumable ~2.3-2.7us pipe latency
    dominate the body; 128KB chunks on the SP HWDGE ring (first-use
    order) beat both SWDGE (+1us latency) and bigger packed transfers
    (completion semaphore fires later).
  - gpsimd tensor ops are ~10x slower than DVE ([128,128] tensor_scalar
    ~2us) - gpsimd only does the diagonal affine_select.
  - ~45 N=64 warm-up matmuls into an unread PSUM tile keep the PE busy
    from ~1.5us into the body so the HAM clock gate (cold 1.2 GHz,
    ~3.4us activity window) flips to 2.4 GHz mid-way through the real
    matmul stream instead of never.
  - folding c1/c3 into the matmul stationaries removes the two
    [128,512] post-reciprocal scales that sat on the output critical
    path (the scalar chain is 3 tiny DVE tensor_scalar ops off-path).
The NRT execution wrapper adds ~1.2us of preamble and ~8us of teardown
(per-engine zeroing of all 256 semaphores) inside the measured window;
that part is runtime-injected and unavoidable.

Matmul operands are bf16 (full-rate PE streaming; fp32 PSUM
accumulation; rel err ~2e-3), everything else fp32. DMAs ride the SP
and ACT HWDGE rings; explicit add_dep_helper edges keep the per-engine
FIFOs free of head-of-line blocking.
"""

import ml_dtypes
import numpy as np

import concourse.tile as tile
from concourse import bacc, mybir
from concourse.bass_utils import run_bass_kernel_spmd
from concourse.tile_rust import add_dep_helper

B = 1024          # number of points
D = 128           # feature dim
NCORES = 8
ROWS = B // NCORES  # output rows per core = 128
H = 512             # column half
F32 = mybir.dt.float32
BF16 = mybir.dt.bfloat16
AF = mybir.ActivationFunctionType
NWARM = 45          # PE warm-up matmuls (HAM un-throttle), N=64 each


def build_nc():
    nc = bacc.Bacc(None)
    xT = nc.declare_dram_parameter("xT", [D, B], BF16, isOutput=False)
    dcT = nc.declare_dram_parameter("dcT", [D, B], BF16, isOutput=False)
    apar = nc.declare_dram_parameter("apar", [D, 1], F32, isOutput=False)
    out = nc.declare_dram_parameter("out", [ROWS, B], F32, isOutput=True)

    with tile.TileContext(nc) as tc:
        with (
            tc.tile_pool(name="big", bufs=1) as big,
            tc.tile_pool(name="small", bufs=1) as small,
            tc.tile_pool(name="ps", bufs=1, space="PSUM") as ps,
        ):
            XB = [big.tile([D, H], BF16, name=f"XB{h}", tag=f"XB{h}") for h in range(2)]
            DCT = [big.tile([D, H], BF16, name=f"DCT{h}", tag=f"DCT{h}") for h in range(2)]
            ET = [big.tile([D, H], BF16, name=f"ET{h}", tag=f"ET{h}") for h in range(2)]
            X2 = [big.tile([D, H], BF16, name=f"X2{h}", tag=f"X2{h}") for h in range(2)]
            SIM = [big.tile([ROWS, H], F32, name=f"SIM{h}", tag=f"SIM{h}") for h in range(2)]
            XSC = small.tile([D, ROWS], BF16, tag="XSC")
            ESC = small.tile([D, ROWS], BF16, tag="ESC")
            X2SC = small.tile([D, ROWS], BF16, tag="X2SC")
            C1ONES = small.tile([D, ROWS], BF16, tag="C1ONES")
            ONESB = small.tile([D, H], BF16, tag="ONESB")
            SA = small.tile([D, 1], F32, tag="SA")
            QS = small.tile([D, 3], F32, tag="QS")
            OB = small.tile([D, 1], F32, tag="OB")
            PS = [ps.tile([ROWS, H], F32, name=f"PS{h}", tag=f"PS{h}") for h in range(2)]
            PSW = ps.tile([ROWS, 64], F32, tag="PSW")

            # ---- input DMAs ----
            # Data halves on the SP HWDGE ring in first-use order; the tiny
            # scalar on the ACT ring (overlaps the ACT table load).
            nc.scalar.dma_start(SA[:], apar[:, :])
            nc.sync.dma_start(XB[0][:], xT[:, 0:H])
            nc.sync.dma_start(DCT[0][:], dcT[:, 0:H])
            nc.sync.dma_start(XB[1][:], xT[:, H:B])
            nc.sync.dma_start(DCT[1][:], dcT[:, H:B])

            # constants (DVE is idle while DMAs land)
            i_ones = nc.vector.memset(ONESB[:], 1.0)
            nc.vector.memset(OB[:], 1.0)

            # ---- PE warm-up: small dummy matmuls into an unread PSUM
            # tile, back-to-back (~53ns apiece) so the HAM activity window
            # is hot by the time the real matmuls stream ----
            i_warm = []
            for k in range(NWARM):
                i_warm.append(nc.tensor.matmul(PSW[:], ONESB[:, 0:ROWS],
                                               ONESB[:, 0:64],
                                               start=True, stop=True))

            # ---- runtime scalars from a (host-replicated to [128,1]) ----
            # c1 = (1-a)/D, c1n2 = -2(1-a)/D, c3 = a/D as [128,1] columns.
            M = mybir.AluOpType.mult
            A = mybir.AluOpType.add
            i_c1 = nc.vector.tensor_scalar(QS[:, 0:1], SA[:, 0:1],
                                           -1.0 / D, 1.0 / D, M, A)
            i_c1n2 = nc.vector.tensor_scalar(QS[:, 1:2], SA[:, 0:1],
                                             2.0 / D, -2.0 / D, M, A)
            i_c3 = nc.vector.tensor_scalar_mul(QS[:, 2:3], SA[:, 0:1],
                                               1.0 / D)

            # ---- DVE feeders (order = DVE FIFO order) ----
            i_xsc = nc.vector.tensor_scalar_mul(XSC[:], XB[0][:, 0:ROWS],
                                                QS[:, 1:2])
            i_x2_0 = nc.vector.tensor_mul(X2[0][:], XB[0][:], XB[0][:])
            i_x2sc = nc.vector.tensor_scalar_mul(X2SC[:], X2[0][:, 0:ROWS],
                                                 QS[:, 0:1])
            i_c1o = nc.vector.tensor_scalar_mul(C1ONES[:], ONESB[:, 0:ROWS],
                                                QS[:, 0:1])

            # ---- ACT: e = exp(1 - dc) -> bf16 ----
            i_exp = [None, None]
            for h in range(2):
                i_exp[h] = nc.scalar.activation(ET[h][:], DCT[h][:], AF.Exp,
                                                bias=OB[:, 0:1], scale=-1.0)

            # ESC ahead of X2[1] on the DVE FIFO: it gates mm2[0] (the
            # E-term close of half 0) while X2[1] only gates mm3[1].
            i_esc = nc.vector.tensor_scalar_mul(ESC[:], ET[0][:, 0:ROWS],
                                                QS[:, 2:3])
            i_x2_1 = nc.vector.tensor_mul(X2[1][:], XB[1][:], XB[1][:])

            # ---- matmul accumulation groups, E term last ----
            i_mm = []
            i_recip = [None, None]
            for h in range(2):
                m1 = nc.tensor.matmul(PS[h][:], XSC[:], XB[h][:],
                                      start=True, stop=False)
                m3 = nc.tensor.matmul(PS[h][:], C1ONES[:], X2[h][:],
                                      start=False, stop=False)
                m4 = nc.tensor.matmul(PS[h][:], X2SC[:], ONESB[:, 0:H],
                                      start=False, stop=False)
                m2 = nc.tensor.matmul(PS[h][:], ESC[:], ET[h][:],
                                      start=False, stop=True)
                i_mm += [m1, m3, m4, m2]
                # ~5x faster than reciprocal(), ~18 correct bits - far inside
                # tolerance; den is bounded away from 0 (>= c3*min(e)^2 > 0)
                i_recip[h] = nc.vector.reciprocal_approx_fast(SIM[h][:],
                                                              PS[h][:])

            # PE program order: warm-ups then group 0 then group 1
            pe_order = i_warm + i_mm
            for a, b in zip(pe_order[1:], pe_order[:-1]):
                add_dep_helper(a.ins, b.ins, sync=False,
                               reason="PE program order")
            # keep the DVE feeders ahead of recip0 on the DVE FIFO
            for dep in (i_x2_0, i_x2sc, i_c1o, i_esc, i_x2_1):
                add_dep_helper(i_recip[0].ins, dep.ins, sync=False,
                               reason="DVE feeders ahead of recip0")

            # diagonal := 1.0 (local columns 0:128 hold the diagonal block)
            nc.gpsimd.affine_select(
                SIM[0][:, 0:ROWS], SIM[0][:, 0:ROWS],
                pattern=[[1, ROWS]], compare_op=mybir.AluOpType.not_equal,
                fill=1.0, base=0, channel_multiplier=-1,
            )

            # out halves on separate rings so the stores overlap
            nc.scalar.dma_start(out[:, 0:H], SIM[0][:])
            nc.sync.dma_start(out[:, H:B], SIM[1][:])
    nc.finalize()
    return nc


def _prep_in_maps(x, dc, dc_param):
    x = np.ascontiguousarray(np.asarray(x, dtype=np.float32))
    dc = np.ascontiguousarray(np.asarray(dc, dtype=np.float32))
    a = np.full((128, 1), np.asarray(dc_param, dtype=np.float32).reshape(()),
                dtype=np.float32)
    xT = np.ascontiguousarray(x.T)
    dcT = np.ascontiguousarray(dc.T)
    in_maps = []
    for c in range(NCORES):
        sh = c * ROWS
        in_maps.append({
            # bf16 transfer format: identical rounding to the on-chip
            # f32->bf16 cast it replaces, at half the HBM traffic
            "xT": np.ascontiguousarray(np.roll(xT, -sh, axis=1)).astype(
                ml_dtypes.bfloat16),
            "dcT": np.ascontiguousarray(np.roll(dcT, -sh, axis=1)).astype(
                ml_dtypes.bfloat16),
            "apar": a,
        })
    return in_maps


def _unshard(results):
    out = np.empty((B, B), dtype=np.float32)
    for c in range(NCORES):
        sh = c * ROWS
        out[sh:sh + ROWS, :] = np.roll(results[c]["out"], sh, axis=1)
    return out


def kernel(x, dc, dc_param):
    nc = build_nc()
    res = run_bass_kernel_spmd(nc, _prep_in_maps(x, dc, dc_param),
                               list(range(NCORES)))
    return _unshard(res.results)


def _ensure_ntff_hook():
    """The agent image's ``antenv`` lacks ``axon_hooks``; synthesize it and
    register the ctypes NTFF-profiling hook so trace=True works."""
    import sys
    import types
    try:
        from antenv.axon_hooks import get_axon_ntff_profile_hook  # noqa: F401
        return
    except ImportError:
        pass
    mod = types.ModuleType("antenv.axon_hooks")
    mod._hook = None

    def set_axon_ntff_profile_hook(h):
        mod._hook = h

    def get_axon_ntff_profile_hook():
        return mod._hook

    mod.set_axon_ntff_profile_hook = set_axon_ntff_profile_hook
    mod.get_axon_ntff_profile_hook = get_axon_ntff_profile_hook
    sys.modules["antenv.axon_hooks"] = mod
    try:
        from trn_agent_boot.trn_boot import _ntff_profile_via_ctypes
        mod._hook = _ntff_profile_via_ctypes("/opt/axon/libaxon_pjrt.so")
    except Exception as e:  # degrade to no-trace
        print(f"ntff hook setup failed: {e}", file=sys.stderr)


def kernel_traced(x, dc, dc_param, reps=3):
    """Like kernel() but captures a neuron-profile trace; returns
    (output, best_exec_time_ns, trace_path). Runs `reps` times (the NEFF is
    compiled once and cached) and reports the fastest - exec time is noisy
    run-to-run (chip power state, co-tenants)."""
    _ensure_ntff_hook()
    nc = build_nc()
    in_maps = _prep_in_maps(x, dc, dc_param)
    best = None
    for _ in range(reps):
        res = run_bass_kernel_spmd(nc, in_maps, list(range(NCORES)),
                                   trace=True,
                                   trace_cores=list(range(NCORES)))
        print(f"  rep exec_time_ns: {res.exec_time_ns}")
        if best is None or (res.exec_time_ns or 1 << 60) < (
                best.exec_time_ns or 1 << 60):
            best = res
    trace_path = None
    if best.instructions_and_trace is not None:
        trace_path = best.instructions_and_trace[1]
    return _unshard(best.results), best.exec_time_ns, trace_path


# revision 9
# speedup vs baseline: 1.3381x; 1.0256x over previous
"""Trainium2 Bass kernel for the pairwise adjacency layer.

Reference math (B=1024 points, D=128 dims):
    a   = dc_param[0]
    e   = exp(1 - dc)                                  # [B, D]
    den[i,j] = mean_d((1-a)*(x[i]-x[j])**2 + a*e[i]*e[j])
    out = 1/den off-diagonal, 1.0 on the diagonal      # [B, B]

Expansion used on-chip (no [B,B,D] tensor ever materializes), with the
runtime scalars folded into the matmul stationaries so PSUM accumulates
den directly and the reciprocal is the last math op:
    den = (-2c1 x_slab)^T @ x       (Gram term)
        + (c1 ones)^T     @ x2      (row broadcast of r_j)
        + (c1 x2_slab)^T  @ ones    (column broadcast of r_i)
        + (c3 e_slab)^T   @ e       (exp cross term)
    c1 = (1-a)/D, c3 = a/D, x2 = x*x, r = sum_d x^2
followed by one DVE reciprocal_approx_fast (~18 correct bits) and a
gpsimd affine_select stamping the diagonal to 1.0.

Sharding: pure output-row-parallel over 8 NeuronCores. Core c computes
output rows [c*128, (c+1)*128). Inputs x^T/dc^T are replicated to every
core (on-chip collectives bounce through HBM and have a ~10us/step
firmware floor - far worse than re-reading 1MB per core). Each core's
copy has its columns rotated left by c*128 so that the diagonal block of
the output is always local columns 0:128, making the diagonal stamp an
SPMD-uniform affine_select; the host unshard rolls each slab back.

Lessons baked in from NTFF traces of prior variants:
  - per-DMA issue ~0.65us + issue-to-consumable ~2.3-2.7us pipe latency
    dominate the body; 128KB chunks on the SP HWDGE ring (first-use
    order) beat both SWDGE (+1us latency) and bigger packed transfers
    (completion semaphore fires later).
  - gpsimd tensor ops are ~10x slower than DVE ([128,128] tensor_scalar
    ~2us) - gpsimd only does the diagonal affine_select.
  - ~45 N=64 warm-up matmuls into an unread PSUM tile keep the PE busy
    from ~1.5us into the body so the HAM clock gate (cold 1.2 GHz,
    ~3.4us activity window) flips to 2.4 GHz mid-way through the real
    matmul stream instead of never.
  - folding c1/c3 into the matmul stationaries removes the two
    [128,512] post-reciprocal scales that sat on the output critical
    path (the scalar chain is 3 tiny DVE tensor_scalar ops off-path).
The NRT execution wrapper adds ~1.2us of preamble and ~8us of teardown
(per-engine zeroing of all 256 semaphores) inside the measured window;
that part is runtime-injected and unavoidable.

Matmul operands are bf16 (full-rate PE streaming; fp32 PSUM
accumulation; rel err ~2e-3), everything else fp32. DMAs ride the SP
and ACT HWDGE rings; explicit add_dep_helper edges keep the per-engine
FIFOs free of head-of-line blocking.
"""

import ml_dtypes
import numpy as np

import concourse.tile as tile
from concourse import bacc, mybir
from concourse.bass_utils import run_bass_kernel_spmd
from concourse.tile_rust import add_dep_helper

B = 1024          # number of points
D = 128           # feature dim
NCORES = 8
ROWS = B // NCORES  # output rows per core = 128
H = 512             # column half
F32 = mybir.dt.float32
BF16 = mybir.dt.bfloat16
AF = mybir.ActivationFunctionType
NWARM = 45          # PE warm-up matmuls (HAM un-throttle), N=64 each


def build_nc():
    nc = bacc.Bacc(None)
    xT = nc.declare_dram_parameter("xT", [D, B], BF16, isOutput=False)
    dcT = nc.declare_dram_parameter("dcT", [D, B], BF16, isOutput=False)
    apar = nc.declare_dram_parameter("apar", [D, 1], F32, isOutput=False)
    out = nc.declare_dram_parameter("out", [ROWS, B], F32, isOutput=True)

    with tile.TileContext(nc) as tc:
        with (
            tc.tile_pool(name="big", bufs=1) as big,
            tc.tile_pool(name="small", bufs=1) as small,
            tc.tile_pool(name="ps", bufs=1, space="PSUM") as ps,
        ):
            XB = [big.tile([D, H], BF16, name=f"XB{h}", tag=f"XB{h}") for h in range(2)]
            DCT = [big.tile([D, H], BF16, name=f"DCT{h}", tag=f"DCT{h}") for h in range(2)]
            ET = [big.tile([D, H], BF16, name=f"ET{h}", tag=f"ET{h}") for h in range(2)]
            X2 = [big.tile([D, H], BF16, name=f"X2{h}", tag=f"X2{h}") for h in range(2)]
            SIM = [big.tile([ROWS, H], F32, name=f"SIM{h}", tag=f"SIM{h}") for h in range(2)]
            XSC = small.tile([D, ROWS], BF16, tag="XSC")
            ESC = small.tile([D, ROWS], BF16, tag="ESC")
            X2SC = small.tile([D, ROWS], BF16, tag="X2SC")
            C1ONES = small.tile([D, ROWS], BF16, tag="C1ONES")
            ONESB = small.tile([D, H], BF16, tag="ONESB")
            SA = small.tile([D, 1], F32, tag="SA")
            QS = small.tile([D, 3], F32, tag="QS")
            OB = small.tile([D, 1], F32, tag="OB")
            PS = [ps.tile([ROWS, H], F32, name=f"PS{h}", tag=f"PS{h}") for h in range(2)]
            PSW = ps.tile([ROWS, 64], F32, tag="PSW")

            # ---- input DMAs ----
            # Data halves on the SP HWDGE ring in first-use order; the tiny
            # scalar on the ACT ring (overlaps the ACT table load).
            nc.scalar.dma_start(SA[:], apar[:, :])
            nc.sync.dma_start(XB[0][:], xT[:, 0:H])
            nc.sync.dma_start(DCT[0][:], dcT[:, 0:H])
            nc.sync.dma_start(XB[1][:], xT[:, H:B])
            nc.sync.dma_start(DCT[1][:], dcT[:, H:B])

            # constants (DVE is idle while DMAs land)
            i_ones = nc.vector.memset(ONESB[:], 1.0)
            nc.vector.memset(OB[:], 1.0)

            # ---- PE warm-up: small dummy matmuls into an unread PSUM
            # tile, back-to-back (~53ns apiece) so the HAM activity window
            # is hot by the time the real matmuls stream ----
            i_warm = []
            for k in range(NWARM):
                i_warm.append(nc.tensor.matmul(PSW[:], ONESB[:, 0:ROWS],
                                               ONESB[:, 0:64],
                                               start=True, stop=True))

            # ---- runtime scalars from a (host-replicated to [128,1]) ----
            # c1 = (1-a)/D, c1n2 = -2(1-a)/D, c3 = a/D as [128,1] columns.
            M = mybir.AluOpType.mult
            A = mybir.AluOpType.add
            i_c1 = nc.vector.tensor_scalar(QS[:, 0:1], SA[:, 0:1],
                                           -1.0 / D, 1.0 / D, M, A)
            i_c1n2 = nc.vector.tensor_scalar(QS[:, 1:2], SA[:, 0:1],
                                             2.0 / D, -2.0 / D, M, A)
            i_c3 = nc.vector.tensor_scalar_mul(QS[:, 2:3], SA[:, 0:1],
                                               1.0 / D)

            # ---- DVE feeders (order = DVE FIFO order) ----
            i_xsc = nc.vector.tensor_scalar_mul(XSC[:], XB[0][:, 0:ROWS],
                                                QS[:, 1:2])
            i_c1o = nc.vector.tensor_scalar_mul(C1ONES[:], ONESB[:, 0:ROWS],
                                                QS[:, 0:1])
            i_x2_0 = nc.vector.tensor_mul(X2[0][:], XB[0][:], XB[0][:])
            i_x2sc = nc.vector.tensor_scalar_mul(X2SC[:], X2[0][:, 0:ROWS],
                                                 QS[:, 0:1])
            # pin the DVE FIFO: small matmul-stationary feeders first (they
            # gate the PE), the big x^2 products after
            add_dep_helper(i_c1o.ins, i_xsc.ins, sync=False,
                           reason="XSC ahead of C1ONES on DVE")
            add_dep_helper(i_x2_0.ins, i_c1o.ins, sync=False,
                           reason="C1ONES ahead of X2[0] on DVE")

            # ---- ACT: e = exp(1 - dc) -> bf16 ----
            i_exp = [None, None]
            for h in range(2):
                i_exp[h] = nc.scalar.activation(ET[h][:], DCT[h][:], AF.Exp,
                                                bias=OB[:, 0:1], scale=-1.0)

            # ESC ahead of X2[1] on the DVE FIFO: it gates mm2[0] (the
            # E-term close of half 0) while X2[1] only gates mm3[1].
            i_esc = nc.vector.tensor_scalar_mul(ESC[:], ET[0][:, 0:ROWS],
                                                QS[:, 2:3])
            i_x2_1 = nc.vector.tensor_mul(X2[1][:], XB[1][:], XB[1][:])

            # ---- matmul accumulation groups, E term last ----
            i_mm = []
            i_recip = [None, None]
            for h in range(2):
                m1 = nc.tensor.matmul(PS[h][:], XSC[:], XB[h][:],
                                      start=True, stop=False)
                m3 = nc.tensor.matmul(PS[h][:], C1ONES[:], X2[h][:],
                                      start=False, stop=False)
                m4 = nc.tensor.matmul(PS[h][:], X2SC[:], ONESB[:, 0:H],
                                      start=False, stop=False)
                m2 = nc.tensor.matmul(PS[h][:], ESC[:], ET[h][:],
                                      start=False, stop=True)
                i_mm += [m1, m3, m4, m2]
                # ~5x faster than reciprocal(), ~18 correct bits - far inside
                # tolerance; den is bounded away from 0 (>= c3*min(e)^2 > 0)
                i_recip[h] = nc.vector.reciprocal_approx_fast(SIM[h][:],
                                                              PS[h][:])

            # PE program order: warm-ups then group 0 then group 1
            pe_order = i_warm + i_mm
            for a, b in zip(pe_order[1:], pe_order[:-1]):
                add_dep_helper(a.ins, b.ins, sync=False,
                               reason="PE program order")
            # keep the DVE feeders ahead of recip0 on the DVE FIFO
            for dep in (i_x2_0, i_x2sc, i_c1o, i_esc, i_x2_1):
                add_dep_helper(i_recip[0].ins, dep.ins, sync=False,
                               reason="DVE feeders ahead of recip0")

            # diagonal := 1.0 (local columns 0:128 hold the diagonal block)
            nc.gpsimd.affine_select(
                SIM[0][:, 0:ROWS], SIM[0][:, 0:ROWS],
                pattern=[[1, ROWS]], compare_op=mybir.AluOpType.not_equal,
                fill=1.0, base=0, channel_multiplier=-1,
            )

            # out halves on separate rings so the stores overlap
            nc.scalar.dma_start(out[:, 0:H], SIM[0][:])
            nc.sync.dma_start(out[:, H:B], SIM[1][:])
    nc.finalize()
    return nc


def _prep_in_maps(x, dc, dc_param):
    x = np.ascontiguousarray(np.asarray(x, dtype=np.float32))
    dc = np.ascontiguousarray(np.asarray(dc, dtype=np.float32))
    a = np.full((128, 1), np.asarray(dc_param, dtype=np.float32).reshape(()),
                dtype=np.float32)
    xT = np.ascontiguousarray(x.T)
    dcT = np.ascontiguousarray(dc.T)
    in_maps = []
    for c in range(NCORES):
        sh = c * ROWS
        in_maps.append({
            # bf16 transfer format: identical rounding to the on-chip
            # f32->bf16 cast it replaces, at half the HBM traffic
            "xT": np.ascontiguousarray(np.roll(xT, -sh, axis=1)).astype(
                ml_dtypes.bfloat16),
            "dcT": np.ascontiguousarray(np.roll(dcT, -sh, axis=1)).astype(
                ml_dtypes.bfloat16),
            "apar": a,
        })
    return in_maps


def _unshard(results):
    out = np.empty((B, B), dtype=np.float32)
    for c in range(NCORES):
        sh = c * ROWS
        out[sh:sh + ROWS, :] = np.roll(results[c]["out"], sh, axis=1)
    return out


def kernel(x, dc, dc_param):
    nc = build_nc()
    res = run_bass_kernel_spmd(nc, _prep_in_maps(x, dc, dc_param),
                               list(range(NCORES)))
    return _unshard(res.results)


def _ensure_ntff_hook():
    """The agent image's ``antenv`` lacks ``axon_hooks``; synthesize it and
    register the ctypes NTFF-profiling hook so trace=True works."""
    import sys
    import types
    try:
        from antenv.axon_hooks import get_axon_ntff_profile_hook  # noqa: F401
        return
    except ImportError:
        pass
    mod = types.ModuleType("antenv.axon_hooks")
    mod._hook = None

    def set_axon_ntff_profile_hook(h):
        mod._hook = h

    def get_axon_ntff_profile_hook():
        return mod._hook

    mod.set_axon_ntff_profile_hook = set_axon_ntff_profile_hook
    mod.get_axon_ntff_profile_hook = get_axon_ntff_profile_hook
    sys.modules["antenv.axon_hooks"] = mod
    try:
        from trn_agent_boot.trn_boot import _ntff_profile_via_ctypes
        mod._hook = _ntff_profile_via_ctypes("/opt/axon/libaxon_pjrt.so")
    except Exception as e:  # degrade to no-trace
        print(f"ntff hook setup failed: {e}", file=sys.stderr)


def kernel_traced(x, dc, dc_param, reps=3):
    """Like kernel() but captures a neuron-profile trace; returns
    (output, best_exec_time_ns, trace_path). Runs `reps` times (the NEFF is
    compiled once and cached) and reports the fastest - exec time is noisy
    run-to-run (chip power state, co-tenants)."""
    _ensure_ntff_hook()
    nc = build_nc()
    in_maps = _prep_in_maps(x, dc, dc_param)
    best = None
    for _ in range(reps):
        res = run_bass_kernel_spmd(nc, in_maps, list(range(NCORES)),
                                   trace=True,
                                   trace_cores=list(range(NCORES)))
        print(f"  rep exec_time_ns: {res.exec_time_ns}")
        if best is None or (res.exec_time_ns or 1 << 60) < (
                best.exec_time_ns or 1 << 60):
            best = res
    trace_path = None
    if best.instructions_and_trace is not None:
        trace_path = best.instructions_and_trace[1]
    return _unshard(best.results), best.exec_time_ns, trace_path
